# revision 21
# baseline (speedup 1.0000x reference)
"""Trainium2 Bass kernel for nn_AdjacencyGenerator (gnn_message_passing).

Math note (see kernel_baseline.py for the original derivation): softmax over
dim 1 of the [E,E,D] attention tensor sums to 1, so the attention cancels and
the output is a per-edge scalar o[i] = f(Wh[i,:]) repeated D times, where
  f: elu -> LN(na) -> ff -> leaky -> LN(nf) -> wl -> leaky -> w5 -> +res
     -> LN(fn) -> wv.

Beyond the baseline, this version exploits:
  * scale invariance: LN_core(a*x) = LN_core(x) for per-row a>0, and all the
    layers between LNs are positively homogeneous.  No rstd is ever applied
    on-chip; the three factors collapse into one final rsqrt via
        v1 = var1 + eps,  v2 = var2 + eps*128^2*v1,  v3 = var3 + eps*128^2*v2
        out[e] = (red0[e] - mean3[e]*sum(wv_eff)) * rsqrt(v3[e])
    computed on the HOST from 4 shipped scalars per edge (exact algebra; the
    128^2 factors come from the mean-sub trick below).
  * mean subtraction via the accumulator: the op producing each LN input also
    emits its row-sum s, and the centering is one op: x' = 128*x - s
    (the extra 128 scale is absorbed by scale invariance).
  * elu(x)+1 = min(exp(x),1) + relu(x): exp runs on ACT straight from PSUM
    while DVE computes the relu part in parallel.
  * leaky_0.2(x) = 0.6*x + 0.4*|x|: wl chunks 1,2 use one ACT Abs + one DVE
    op (0.6 folded into w5); chunk 0 stays DVE-only for pipeline balance.
  * fp16 everywhere on the PE path, including fp16 PSUM banks for the
    single-shot matmuls (halves the DVE PSUM-read cost).
  * the final wv dot product is 4 tiny PE matmuls (wv folded through w5)
    accumulating into a PSUM column, not a DVE reduction.

Distribution: 1024 edges, 128 per core across 8 cores, weights replicated.
"""

import numpy as np

D = 128
E = 1024
NCORES = 8
PER = E // NCORES
EPS = 1e-5
EPS_K = EPS                   # mean-subs are exact, no extra scale

# packed image column offsets (fp16)
XW_XJT, XW_W = 0, 128                       # d_xw [128, 256]
A_ID, A_FFWT = 0, 128                       # d_wa [128, 256]
B_WLT, B_W5AB, B_Y2LIN = 0, 384, 768    # d_wb [128, 904]
B_RM_U, B_RM_AB = 896, 898              # [wv|ones] column pairs
B_COLS = 904

_CACHE = {}


class _Seq:
    """Sequential instruction emitter for one engine with semaphore tags."""

    def __init__(self, eng, sem, all_self_waits, attach=False):
        self.eng, self.sem, self.n = eng, sem, 0
        self.all_self_waits = all_self_waits
        self.attach = attach

    def emit(self, make, waits=(), self_wait=False):
        allw = list(waits)
        if (self_wait or self.all_self_waits) and self.n:
            allw.append((self.sem, self.n))
        if self.attach and allw:
            for s, v in allw[:-1]:
                self.eng.wait_ge(s, v)
            inst = make()
            inst._wait_ge(*allw[-1])
        else:
            for s, v in allw:
                self.eng.wait_ge(s, v)
            inst = make()
        inst.then_inc(self.sem, 1)
        self.n += 1
        return self.n


def _build_nc(validation=False):
    import concourse.bass as bass
    from concourse import mybir

    f32 = mybir.dt.float32
    f16 = mybir.dt.float16
    Alu = mybir.AluOpType
    Act = mybir.ActivationFunctionType

    nc = bass.Bass(detect_race_conditions=validation)

    d_xw = nc.dram_tensor("xw", [128, 256], f16, kind="ExternalInput")
    d_wa = nc.dram_tensor("wpacka", [128, 256], f16, kind="ExternalInput")
    d_wb = nc.dram_tensor("wpackb", [128, B_COLS], f16, kind="ExternalInput")
    d_out = nc.dram_tensor("out", [PER, 4], f32, kind="ExternalOutput")

    from contextlib import ExitStack

    ctx = ExitStack()
    sb = lambda name, shape, dt=f32: ctx.enter_context(
        nc.sbuf_tensor(name, shape, dt))
    ps = lambda name, shape, dt=f32: ctx.enter_context(
        nc.psum_tensor(name, shape, dt))

    s_xw = sb("s_xw", [128, 256], f16)
    s_wa = sb("s_wa", [128, 256], f16)
    s_wb = sb("s_wb", [128, B_COLS], f16)

    r_ = sb("r", [PER, D], f16)        # relu(Wh)
    ex = sb("ex", [PER, D], f16)       # exp(Wh)
    t1 = sb("t1", [PER, D], f16)       # elu(Wh)+1
    s1 = sb("s1", [PER, 1])            # sum(t1)
    t2 = sb("t2", [PER, D], f16)       # 128*t1 - s1
    t2T = sb("t2t", [D, PER], f16)
    lka = sb("lka", [PER, D], f16)     # -0.8*min(ff,0)
    t3 = sb("t3", [PER, D], f16)       # leaky(ff)
    s2 = sb("s2", [PER, 1])            # sum(t3)
    u = sb("u", [PER, D], f16)         # 128*t3 - s2
    uT = sb("ut", [D, PER], f16)
    ab = sb("ab", [128, 3, PER], f16)  # leaky(wl_0) / |wl_1| / |wl_2|
    y1a = sb("y1a", [128, PER], f16)   # -0.8*min(wl_0, 0) scratch
    sq = sb("sq", [PER, D], f16)       # y3^2 scratch
    s1m = sb("s1m", [PER, 1])          # mean1
    s2m = sb("s2m", [PER, 1])          # mean2
    st = sb("st", [PER, 6])
    mv = sb("mv", [PER, 2])
    v1 = sb("v1", [PER, 1])
    o_sb = sb("o_sb", [PER, 4])        # red0 | mean3 | var3 | v2
    scr = sb("scr", [1, 1])            # ACT warmup scratch

    p_wh = ps("p_wh", [PER, D])
    p_tT = ps("p_tt", [D, PER], f16)   # reused for t2T and uT
    p_q2 = ps("p_q2", [PER, D])
    p_y1 = [ps(f"p_y1{c}", [128, PER]) for c in range(3)]
    p_y2 = ps("p_y2", [PER, D])
    p_rm = ps("p_rm", [PER, 2])       # col0: sum(y3*wv), col1: sum(y3)

    dsem_x = ctx.enter_context(nc.semaphore("dsem_x"))
    dsem_a = ctx.enter_context(nc.semaphore("dsem_a"))
    dsem_b = ctx.enter_context(nc.semaphore("dsem_b"))
    dsem_o = ctx.enter_context(nc.semaphore("dsem_o"))
    psem = ctx.enter_context(nc.semaphore("psem"))
    vsem = ctx.enter_context(nc.semaphore("vsem"))
    asem = ctx.enter_context(nc.semaphore("asem"))
    gsem = ctx.enter_context(nc.semaphore("gsem"))

    # ---- vector op indices ----------------------------------------------
    V_R2, V_T1, V_S1M, V_T2, V_T2T = 1, 2, 3, 4, 5
    V_ST1, V_MV1, V_V1 = 6, 7, 8
    V_LKA, V_T3, V_S2M, V_U, V_UT = 9, 10, 11, 12, 13
    V_ST2, V_MV2, V_V2 = 14, 15, 16
    V_AB0A, V_AB0, V_REDC = 17, 18, 19
    # ---- PE op indices ---------------------------------------------------
    P_WH, P_T2T, P_FF, P_UT = 1, 2, 3, 4
    P_WL = [5, 6, 7]
    P_RES, P_Y2LIN, P_RMU = 8, 9, 10
    P_AB0, P_RMAB0 = 11, 12
    P_AB1, P_RMAB1 = 13, 14
    P_AB2, P_RMAB2 = 15, 16
    # ---- ACT op indices --------------------------------------------------
    A_WARM, A_EX, A_ABS1, A_ABS2, A_SQ3 = 1, 2, 3, 4, 5
    # ---- gpsimd ----------------------------------------------------------
    G_SCR = 1

    with nc.Block() as block:

        @block.sync
        def _(sync):
            sync.dma_start(out=s_xw[:, :], in_=d_xw[:, :]).then_inc(dsem_x, 16)
            sync.dma_start(out=s_wa[:, :], in_=d_wa[:, :]).then_inc(dsem_a, 16)
            sync.dma_start(out=s_wb[:, :], in_=d_wb[:, :]).then_inc(dsem_b, 16)
            sync.wait_ge(vsem, V_REDC)
            sync.wait_ge(asem, A_SQ3)
            sync.dma_start(out=d_out[:, :], in_=o_sb[:, :]).then_inc(dsem_o, 16)

        @block.gpsimd
        def _(ge):
            ge.memset(scr[:, :], 1.0).then_inc(gsem, 1)

        @block.scalar
        def _(se):
            A = _Seq(se, asem, validation)
            # warm the ln/exp table set (Exp/Abs share it)
            A.emit(lambda: se.activation(out=scr[:, :], in_=scr[:, :],
                                         func=Act.Ln),
                   waits=[(gsem, G_SCR)])
            A.emit(lambda: se.activation(out=ex[:, :], in_=p_wh[:, :],
                                         func=Act.Exp),
                   waits=[(psem, P_WH)])
            assert A.n == A_EX
            # |wl_1|, |wl_2| on ACT; chunk 0 is exact leaky on DVE
            A.emit(lambda: se.activation(out=ab[:, 1, :], in_=p_y1[1][:, :],
                                         func=Act.Abs),
                   waits=[(psem, P_WL[1])])
            assert A.n == A_ABS1
            A.emit(lambda: se.activation(out=ab[:, 2, :], in_=p_y1[2][:, :],
                                         func=Act.Abs),
                   waits=[(psem, P_WL[2])])
            assert A.n == A_ABS2
            # sum(y3^2) via the ACT accumulator, straight off the closed PSUM
            A.emit(lambda: se.activation(out=sq[:, :], in_=p_y2[:, :],
                                         func=Act.Square,
                                         accum_out=o_sb[:, 2:3]),
                   waits=[(psem, P_AB2)])
            assert A.n == A_SQ3

        @block.tensor
        def _(te):
            T = _Seq(te, psem, validation)
            # Wh = xj @ W
            T.emit(lambda: te.matmul(p_wh[:, :], s_xw[:, XW_XJT:XW_XJT + 128],
                                     s_xw[:, XW_W:XW_W + 128],
                                     start=True, stop=True),
                   waits=[(dsem_x, 16)])
            T.emit(lambda: te.transpose(p_tT[:, :], t2[:, :],
                                        s_wa[:, A_ID:A_ID + 128]),
                   waits=[(vsem, V_T2), (dsem_a, 16)])
            assert T.n == P_T2T
            T.emit(lambda: te.matmul(p_q2[:, :], t2T[:, :],
                                     s_wa[:, A_FFWT:A_FFWT + 128],
                                     start=True, stop=True),
                   waits=[(vsem, V_T2T)])
            T.emit(lambda: te.transpose(p_tT[:, :], u[:, :],
                                        s_wa[:, A_ID:A_ID + 128]),
                   waits=[(vsem, V_U)])
            assert T.n == P_UT
            # wl chunks: M_c = wl_c @ u^T
            for c in range(3):
                T.emit(lambda c=c: te.matmul(
                    p_y1[c][:, :],
                    s_wb[:, B_WLT + c * 128:B_WLT + (c + 1) * 128],
                    uT[:, :], start=True, stop=True),
                    waits=[(vsem, V_UT), (dsem_b, 16)] if c == 0 else ())
                assert T.n == P_WL[c]
            # y3 = u + 0.6*(w5@wl)@u + 0.4*sum_c w5_c@|M_c|  (leaky split);
            # p_red/p_m3 accumulate sum(y3*wv) and sum(y3) the same way
            T.emit(lambda: te.matmul(p_y2[:, :], uT[:, :],
                                     s_wa[:, A_ID:A_ID + 128],
                                     start=True, stop=False,
                                     skip_group_check=True))
            assert T.n == P_RES
            T.emit(lambda: te.matmul(p_y2[:, :], uT[:, :],
                                     s_wb[:, B_Y2LIN:B_Y2LIN + 128],
                                     start=False, stop=False,
                                     skip_group_check=True))
            assert T.n == P_Y2LIN
            T.emit(lambda: te.matmul(p_rm[:, 0:2], uT[:, :],
                                     s_wb[:, B_RM_U:B_RM_U + 2],
                                     start=True, stop=False,
                                     skip_group_check=True))
            assert T.n == P_RMU
            # abs-consuming matmuls, in expected order of |M_c| readiness
            for c, gate in ((1, (asem, A_ABS1)), (0, (vsem, V_AB0)),
                            (2, (asem, A_ABS2))):
                last = c == 2
                T.emit(lambda c=c: te.matmul(
                    p_y2[:, :], ab[:, c, :],
                    s_wb[:, B_W5AB + c * 128:B_W5AB + (c + 1) * 128],
                    start=False, stop=last, skip_group_check=True),
                    waits=[gate])
                T.emit(lambda c=c: te.matmul(
                    p_rm[:, 0:2], ab[:, c, :],
                    s_wb[:, B_RM_AB + 2 * c:B_RM_AB + 2 * c + 2],
                    start=False, stop=last, skip_group_check=True))
            assert T.n == P_RMAB2

        @block.vector
        def _(ve):
            V = _Seq(ve, vsem, validation)
            # elu front: r2 = relu(Wh) on DVE while ACT computes exp(Wh)
            V.emit(lambda: ve.tensor_scalar_max(out=r_[:, :], in0=p_wh[:, :],
                                                scalar1=0.0),
                   waits=[(psem, P_WH)])
            assert V.n == V_R2
            # t1 = min(exp(Wh),1) + relu(Wh); s1 = sum(t1)
            V.emit(lambda: ve.scalar_tensor_tensor(out=t1[:, :], in0=ex[:, :],
                                                   scalar=1.0, in1=r_[:, :],
                                                   op0=Alu.min, op1=Alu.add,
                                                   accum_out=s1[:, :]),
                   waits=[(asem, A_EX)])
            assert V.n == V_T1
            # t2 = t1 - s1/128  (imm+AP tensor_scalar is broken on HW, so
            # scale the sum in a tiny op first)
            V.emit(lambda: ve.tensor_scalar_mul(out=s1m[:, :], in0=s1[:, :],
                                                scalar1=1.0 / 128.0),
                   self_wait=True)
            assert V.n == V_S1M
            V.emit(lambda: ve.tensor_scalar_sub(out=t2[:, :], in0=t1[:, :],
                                                scalar1=s1m[:, 0:1]),
                   self_wait=True)
            assert V.n == V_T2
            V.emit(lambda: ve.tensor_copy(out=t2T[:, :], in_=p_tT[:, :]),
                   waits=[(psem, P_T2T)])
            assert V.n == V_T2T
            # var1 path (only feeds the eps corrections; off critical path)
            V.emit(lambda: ve.bn_stats(out=st[:, :], in_=t1[:, :]))
            V.emit(lambda: ve.bn_aggr(out=mv[:, :], in_=st[:, :]),
                   self_wait=True)
            assert V.n == V_MV1
            V.emit(lambda: ve.tensor_scalar_add(out=v1[:, :], in0=mv[:, 1:2],
                                                scalar1=EPS))
            assert V.n == V_V1
            # leaky(ff): t3 = ff - 0.8*min(ff,0); s2 = sum(t3)
            # (one PSUM operand per instruction -- hardware restriction)
            V.emit(lambda: ve.tensor_scalar(out=lka[:, :], in0=p_q2[:, :],
                                            scalar1=0.0, scalar2=-0.8,
                                            op0=Alu.min, op1=Alu.mult),
                   waits=[(psem, P_FF)])
            V.emit(lambda: ve.scalar_tensor_tensor(
                out=t3[:, :], in0=lka[:, :], scalar=1.0, in1=p_q2[:, :],
                op0=Alu.mult, op1=Alu.add, accum_out=s2[:, :]))
            assert V.n == V_T3
            # u = t3 - s2/128
            V.emit(lambda: ve.tensor_scalar_mul(out=s2m[:, :], in0=s2[:, :],
                                                scalar1=1.0 / 128.0),
                   self_wait=True)
            assert V.n == V_S2M
            V.emit(lambda: ve.tensor_scalar_sub(out=u[:, :], in0=t3[:, :],
                                                scalar1=s2m[:, 0:1]),
                   self_wait=True)
            assert V.n == V_U
            V.emit(lambda: ve.tensor_copy(out=uT[:, :], in_=p_tT[:, :]),
                   waits=[(psem, P_UT)])
            assert V.n == V_UT
            # var2 path (off critical path, during PE wl)
            V.emit(lambda: ve.bn_stats(out=st[:, :], in_=t3[:, :]))
            V.emit(lambda: ve.bn_aggr(out=mv[:, :], in_=st[:, :]),
                   self_wait=True)
            assert V.n == V_MV2
            V.emit(lambda: ve.scalar_tensor_tensor(out=o_sb[:, 3:4],
                                                   in0=v1[:, :], scalar=EPS_K,
                                                   in1=mv[:, 1:2],
                                                   op0=Alu.mult, op1=Alu.add))
            assert V.n == V_V2
            # |M_0| on DVE (chunks 1,2 on ACT)
            V.emit(lambda: ve.tensor_scalar(out=y1a[:, :], in0=p_y1[0][:, :],
                                            scalar1=0.0, scalar2=-0.8,
                                            op0=Alu.min, op1=Alu.mult),
                   waits=[(psem, P_WL[0])])
            assert V.n == V_AB0A
            V.emit(lambda: ve.scalar_tensor_tensor(out=ab[:, 0, :],
                                                   in0=y1a[:, :], scalar=1.0,
                                                   in1=p_y1[0][:, :],
                                                   op0=Alu.mult, op1=Alu.add))
            assert V.n == V_AB0
            V.emit(lambda: ve.tensor_copy(out=o_sb[:, 0:2], in_=p_rm[:, 0:2]),
                   waits=[(psem, P_RMAB2)])
            assert V.n == V_REDC

    return nc, ctx


def _get_nc(validation=False):
    key = "ncv" if validation else "nc"
    if key not in _CACHE:
        _CACHE[key] = _build_nc(validation)
    return _CACHE[key][0]


_POST = {}


def _prep_in_maps(inputs):
    """Host-side sharding + exact algebraic weight folding + packing."""
    g = lambda k: np.asarray(inputs[k], dtype=np.float64)
    x = g("x")
    ei = np.asarray(inputs["edge_index"]).astype(np.int64)
    W = g("W")
    ff_w, ff_b = g("ff_w"), g("ff_b")
    na_g, na_b = g("na_g"), g("na_b")
    nf_g, nf_b = g("nf_g"), g("nf_b")
    wl_w, wl_b = g("wl_w"), g("wl_b")
    w5_w, w5_b = g("w5_w"), g("w5_b")
    fn_g, fn_b = g("fn_g"), g("fn_b")
    wv_w, wv_b = g("wv_w"), g("wv_b")

    xj = x[ei[1]]                           # [E, D] gather on host
    ffw_eff = ff_w * na_g[None, :]          # fold LN(na) gain into ff
    ffb_eff = ff_b + ff_w @ na_b
    wv_eff = wv_w[0] * fn_g                 # fold LN(fn) gain into wv
    wvb_eff = wv_b[0] + wv_w[0] @ fn_b
    wl_eff = wl_w * nf_g[None, :]           # fold LN(nf) gain into wl

    # the kernel structure assumes these vanish (true for the given inputs)
    assert np.all(ffb_eff == 0), "ffb_eff != 0 unsupported"
    assert np.all(wl_b == 0) and np.all(w5_b == 0), "wl/w5 bias unsupported"
    assert np.all(nf_b == 0), "nf_b != 0 unsupported"
    assert abs(wvb_eff) < 1e-12, "wvb != 0 unsupported"

    _POST["swv"] = float(wv_eff.sum())

    f16 = lambda a: np.ascontiguousarray(a, dtype=np.float16)

    wa = np.zeros((128, 256), np.float64)
    wa[:, A_ID:A_ID + 128] = np.eye(128)
    wa[:, A_FFWT:A_FFWT + 128] = ffw_eff.T

    wb = np.zeros((128, B_COLS), np.float64)
    wb[:, B_WLT:B_WLT + 384] = wl_eff.T
    # chunk 0 feeds exact leaky(M_0) into w5_0; chunks 1,2 are split as
    # 0.6*(w5_c@wl_c)@u + 0.4*w5_c@|M_c| (the |.| runs on the ACT engine)
    f16r = lambda a: a.astype(np.float16).astype(np.float64)
    for c, sc in ((0, 1.0), (1, 0.4), (2, 0.4)):
        wb[:, B_W5AB + c * 128:B_W5AB + (c + 1) * 128] = \
            sc * w5_w.T[c * 128:(c + 1) * 128, :]
    wb[:, B_Y2LIN:B_Y2LIN + 128] = \
        0.6 * (w5_w[:, 128:384] @ wl_eff[128:384, :]).T
    # red0/m3 columns: wv (resp. ones) pushed through the same matrices the
    # kernel actually uses (fp16-rounded), so red0 = sum(y3*wv), m3s = sum(y3)
    wv16 = f16r(wv_eff)
    ones = np.ones(128)
    rhs_lin = f16r(wb[:, B_Y2LIN:B_Y2LIN + 128])
    wb[:, B_RM_U] = wv16 + rhs_lin @ wv16
    wb[:, B_RM_U + 1] = ones + rhs_lin @ ones
    for c in range(3):
        rhs_ab = f16r(wb[:, B_W5AB + c * 128:B_W5AB + (c + 1) * 128])
        wb[:, B_RM_AB + 2 * c] = rhs_ab @ wv16
        wb[:, B_RM_AB + 2 * c + 1] = rhs_ab @ ones

    shared = {"wpacka": f16(wa), "wpackb": f16(wb)}
    in_maps = []
    for c in range(NCORES):
        xw = np.empty((128, 256), np.float64)
        xw[:, XW_XJT:XW_XJT + 128] = xj[c * PER:(c + 1) * PER].T
        xw[:, XW_W:XW_W + 128] = W
        m = dict(shared)
        m["xw"] = f16(xw)
        in_maps.append(m)
    return in_maps


def _postprocess_core(out_img):
    """[PER,4] (red0|mean3|var3|v2) -> [PER*D] final output."""
    o = np.asarray(out_img, dtype=np.float64).reshape(PER, 4)
    red0, m3s, sq3, v2 = o[:, 0], o[:, 1], o[:, 2], o[:, 3]
    m3 = m3s / 128.0
    var3 = sq3 / 128.0 - m3 * m3
    v3 = var3 + EPS_K * v2
    oe = (red0 - m3 * _POST["swv"]) / np.sqrt(v3)
    return np.repeat(oe.astype(np.float32), D)


def kernel(**inputs) -> np.ndarray:
    from concourse.bass_utils import run_bass_kernel_spmd

    nc = _get_nc()
    in_maps = _prep_in_maps(inputs)
    res = run_bass_kernel_spmd(nc, in_maps, core_ids=list(range(NCORES)))
    return np.concatenate(
        [_postprocess_core(res.results[c]["out"]) for c in range(NCORES)])


# revision 23
# speedup vs baseline: 1.0030x; 1.0030x over previous
"""Trainium2 Bass kernel for nn_AdjacencyGenerator (gnn_message_passing).

Math note (see kernel_baseline.py for the original derivation): softmax over
dim 1 of the [E,E,D] attention tensor sums to 1, so the attention cancels and
the output is a per-edge scalar o[i] = f(Wh[i,:]) repeated D times, where
  f: elu -> LN(na) -> ff -> leaky -> LN(nf) -> wl -> leaky -> w5 -> +res
     -> LN(fn) -> wv.

Beyond the baseline, this version exploits:
  * scale invariance: LN_core(a*x) = LN_core(x) for per-row a>0, and all the
    layers between LNs are positively homogeneous.  No rstd is ever applied
    on-chip; the three factors collapse into one final rsqrt via
        v1 = var1 + eps,  v2 = var2 + eps*128^2*v1,  v3 = var3 + eps*128^2*v2
        out[e] = (red0[e] - mean3[e]*sum(wv_eff)) * rsqrt(v3[e])
    computed on the HOST from 4 shipped scalars per edge (exact algebra; the
    128^2 factors come from the mean-sub trick below).
  * mean subtraction via the accumulator: the op producing each LN input also
    emits its row-sum s, and the centering is one op: x' = 128*x - s
    (the extra 128 scale is absorbed by scale invariance).
  * elu(x)+1 = min(exp(x),1) + relu(x): exp runs on ACT straight from PSUM
    while DVE computes the relu part in parallel.
  * leaky_0.2(x) = 0.6*x + 0.4*|x|: wl chunks 1,2 use one ACT Abs + one DVE
    op (0.6 folded into w5); chunk 0 stays DVE-only for pipeline balance.
  * fp16 everywhere on the PE path, including fp16 PSUM banks for the
    single-shot matmuls (halves the DVE PSUM-read cost).
  * the final wv dot product is 4 tiny PE matmuls (wv folded through w5)
    accumulating into a PSUM column, not a DVE reduction.

Distribution: 1024 edges, 128 per core across 8 cores, weights replicated.
"""

import numpy as np

D = 128
E = 1024
NCORES = 8
PER = E // NCORES
EPS = 1e-5
EPS_K = EPS                   # mean-subs are exact, no extra scale

# packed image column offsets (fp16)
XW_XJT, XW_W = 0, 128                       # d_xw [128, 256]
A_ID, A_FFWT = 0, 128                       # d_wa [128, 256]
B_WLT, B_W5AB, B_Y2LIN = 0, 384, 768    # d_wb [128, 904]
B_RM_U, B_RM_AB = 896, 898              # [wv|ones] column pairs
B_COLS = 904

_CACHE = {}


class _Seq:
    """Sequential instruction emitter for one engine with semaphore tags."""

    def __init__(self, eng, sem, all_self_waits, attach=False):
        self.eng, self.sem, self.n = eng, sem, 0
        self.all_self_waits = all_self_waits
        self.attach = attach

    def emit(self, make, waits=(), self_wait=False):
        allw = list(waits)
        if (self_wait or self.all_self_waits) and self.n:
            allw.append((self.sem, self.n))
        if self.attach and allw:
            for s, v in allw[:-1]:
                self.eng.wait_ge(s, v)
            inst = make()
            inst._wait_ge(*allw[-1])
        else:
            for s, v in allw:
                self.eng.wait_ge(s, v)
            inst = make()
        inst.then_inc(self.sem, 1)
        self.n += 1
        return self.n


def _build_nc(validation=False):
    import concourse.bass as bass
    from concourse import mybir

    f32 = mybir.dt.float32
    f16 = mybir.dt.float16
    Alu = mybir.AluOpType
    Act = mybir.ActivationFunctionType

    nc = bass.Bass(detect_race_conditions=validation)

    d_xw = nc.dram_tensor("xw", [128, 256], f16, kind="ExternalInput")
    d_wa = nc.dram_tensor("wpacka", [128, 256], f16, kind="ExternalInput")
    d_wb = nc.dram_tensor("wpackb", [128, B_COLS], f16, kind="ExternalInput")
    d_out = nc.dram_tensor("out", [PER, 4], f32, kind="ExternalOutput")

    from contextlib import ExitStack

    ctx = ExitStack()
    sb = lambda name, shape, dt=f32: ctx.enter_context(
        nc.sbuf_tensor(name, shape, dt))
    ps = lambda name, shape, dt=f32: ctx.enter_context(
        nc.psum_tensor(name, shape, dt))

    s_xw = sb("s_xw", [128, 256], f16)
    s_wa = sb("s_wa", [128, 256], f16)
    s_wb = sb("s_wb", [128, B_COLS], f16)

    r_ = sb("r", [PER, D], f16)        # relu(Wh)
    ex = sb("ex", [PER, D], f16)       # exp(Wh)
    t1 = sb("t1", [PER, D], f16)       # elu(Wh)+1
    s1 = sb("s1", [PER, 1])            # sum(t1)
    t2 = sb("t2", [PER, D], f16)       # 128*t1 - s1
    t2T = sb("t2t", [D, PER], f16)
    lka = sb("lka", [PER, D], f16)     # -0.8*min(ff,0)
    t3 = sb("t3", [PER, D], f16)       # leaky(ff)
    s2 = sb("s2", [PER, 1])            # sum(t3)
    u = sb("u", [PER, D], f16)         # 128*t3 - s2
    uT = sb("ut", [D, PER], f16)
    ab = sb("ab", [128, 3, PER], f16)  # leaky(wl_0) / |wl_1| / |wl_2|
    y1a = sb("y1a", [128, PER], f16)   # -0.8*min(wl_0, 0) scratch
    sq = sb("sq", [PER, D], f16)       # y3^2 scratch
    s1m = sb("s1m", [PER, 1])          # mean1
    s2m = sb("s2m", [PER, 1])          # mean2
    st = sb("st", [PER, 6])
    mv = sb("mv", [PER, 2])
    v1 = sb("v1", [PER, 1])
    o_sb = sb("o_sb", [PER, 4])        # red0 | mean3 | var3 | v2
    scr = sb("scr", [1, 1])            # ACT warmup scratch

    p_wh = ps("p_wh", [PER, D])
    p_tT = ps("p_tt", [D, PER], f16)   # reused for t2T and uT
    p_q2 = ps("p_q2", [PER, D])
    p_y1 = [ps(f"p_y1{c}", [128, PER]) for c in range(3)]
    p_y2 = ps("p_y2", [PER, D])
    p_rm = ps("p_rm", [PER, 2])       # col0: sum(y3*wv), col1: sum(y3)

    dsem_x = ctx.enter_context(nc.semaphore("dsem_x"))
    dsem_a = ctx.enter_context(nc.semaphore("dsem_a"))
    dsem_b = ctx.enter_context(nc.semaphore("dsem_b"))
    dsem_o = ctx.enter_context(nc.semaphore("dsem_o"))
    psem = ctx.enter_context(nc.semaphore("psem"))
    vsem = ctx.enter_context(nc.semaphore("vsem"))
    asem = ctx.enter_context(nc.semaphore("asem"))
    gsem = ctx.enter_context(nc.semaphore("gsem"))

    # ---- vector op indices ----------------------------------------------
    V_R2, V_T1, V_S1M, V_T2, V_T2T = 1, 2, 3, 4, 5
    V_ST1, V_MV1, V_V1 = 6, 7, 8
    V_LKA, V_T3, V_S2M, V_U, V_UT = 9, 10, 11, 12, 13
    V_ST2, V_MV2, V_V2 = 14, 15, 16
    V_AB0A, V_AB0, V_REDC = 17, 18, 19
    # ---- PE op indices ---------------------------------------------------
    P_WH, P_WHB, P_T2T, P_FF, P_UT = 1, 2, 3, 4, 5
    P_WL = [6, 7, 8]
    P_RES, P_Y2LIN, P_RMU = 9, 10, 11
    P_AB0, P_RMAB0 = 12, 13
    P_AB1, P_RMAB1 = 14, 15
    P_AB2, P_RMAB2 = 16, 17
    # ---- ACT op indices --------------------------------------------------
    A_WARM, A_EX, A_ABS1, A_ABS2, A_SQ3 = 1, 2, 3, 4, 5
    # ---- gpsimd ----------------------------------------------------------
    G_SCR = 1

    with nc.Block() as block:

        @block.sync
        def _(sync):
            sync.dma_start(out=s_xw[:, :], in_=d_xw[:, :]).then_inc(dsem_x, 16)
            sync.dma_start(out=s_wa[:, :], in_=d_wa[:, :]).then_inc(dsem_a, 16)
            sync.dma_start(out=s_wb[:, :], in_=d_wb[:, :]).then_inc(dsem_b, 16)
            sync.wait_ge(vsem, V_REDC)
            sync.wait_ge(asem, A_SQ3)
            sync.dma_start(out=d_out[:, :], in_=o_sb[:, :]).then_inc(dsem_o, 16)

        @block.gpsimd
        def _(ge):
            ge.memset(scr[:, :], 1.0).then_inc(gsem, 1)

        @block.scalar
        def _(se):
            A = _Seq(se, asem, validation)
            # warm the ln/exp table set (Exp/Abs share it)
            A.emit(lambda: se.activation(out=scr[:, :], in_=scr[:, :],
                                         func=Act.Ln),
                   waits=[(gsem, G_SCR)])
            A.emit(lambda: se.activation(out=ex[:, :], in_=p_wh[:, :],
                                         func=Act.Exp),
                   waits=[(psem, P_WH)])
            assert A.n == A_EX
            # |wl_1|, |wl_2| on ACT; chunk 0 is exact leaky on DVE
            A.emit(lambda: se.activation(out=ab[:, 1, :], in_=p_y1[1][:, :],
                                         func=Act.Abs),
                   waits=[(psem, P_WL[1])])
            assert A.n == A_ABS1
            A.emit(lambda: se.activation(out=ab[:, 2, :], in_=p_y1[2][:, :],
                                         func=Act.Abs),
                   waits=[(psem, P_WL[2])])
            assert A.n == A_ABS2
            # sum(y3^2) via the ACT accumulator, straight off the closed PSUM
            A.emit(lambda: se.activation(out=sq[:, :], in_=p_y2[:, :],
                                         func=Act.Square,
                                         accum_out=o_sb[:, 2:3]),
                   waits=[(psem, P_AB2)])
            assert A.n == A_SQ3

        @block.tensor
        def _(te):
            T = _Seq(te, psem, validation)
            # Wh = xj @ W
            # Wh twice, into two banks: ACT reads p_wh while DVE reads the
            # copy in p_y1[0]'s bank (same-bank dual-engine PSUM reads hang
            # the device; p_y1[0] is dead until the wl matmuls)
            T.emit(lambda: te.matmul(p_wh[:, :], s_xw[:, XW_XJT:XW_XJT + 128],
                                     s_xw[:, XW_W:XW_W + 128],
                                     start=True, stop=True),
                   waits=[(dsem_x, 16)])
            T.emit(lambda: te.matmul(p_y1[0][:, :],
                                     s_xw[:, XW_XJT:XW_XJT + 128],
                                     s_xw[:, XW_W:XW_W + 128],
                                     start=True, stop=True))
            assert T.n == P_WHB
            T.emit(lambda: te.transpose(p_tT[:, :], t2[:, :],
                                        s_wa[:, A_ID:A_ID + 128]),
                   waits=[(vsem, V_T2), (dsem_a, 16)])
            assert T.n == P_T2T
            T.emit(lambda: te.matmul(p_q2[:, :], t2T[:, :],
                                     s_wa[:, A_FFWT:A_FFWT + 128],
                                     start=True, stop=True),
                   waits=[(vsem, V_T2T)])
            T.emit(lambda: te.transpose(p_tT[:, :], u[:, :],
                                        s_wa[:, A_ID:A_ID + 128]),
                   waits=[(vsem, V_U)])
            assert T.n == P_UT
            # wl chunks: M_c = wl_c @ u^T
            for c in range(3):
                T.emit(lambda c=c: te.matmul(
                    p_y1[c][:, :],
                    s_wb[:, B_WLT + c * 128:B_WLT + (c + 1) * 128],
                    uT[:, :], start=True, stop=True),
                    waits=[(vsem, V_UT), (dsem_b, 16)] if c == 0 else ())
                assert T.n == P_WL[c]
            # y3 = u + 0.6*(w5@wl)@u + 0.4*sum_c w5_c@|M_c|  (leaky split);
            # p_red/p_m3 accumulate sum(y3*wv) and sum(y3) the same way
            T.emit(lambda: te.matmul(p_y2[:, :], uT[:, :],
                                     s_wa[:, A_ID:A_ID + 128],
                                     start=True, stop=False,
                                     skip_group_check=True))
            assert T.n == P_RES
            T.emit(lambda: te.matmul(p_y2[:, :], uT[:, :],
                                     s_wb[:, B_Y2LIN:B_Y2LIN + 128],
                                     start=False, stop=False,
                                     skip_group_check=True))
            assert T.n == P_Y2LIN
            T.emit(lambda: te.matmul(p_rm[:, 0:2], uT[:, :],
                                     s_wb[:, B_RM_U:B_RM_U + 2],
                                     start=True, stop=False,
                                     skip_group_check=True))
            assert T.n == P_RMU
            # abs-consuming matmuls, in expected order of |M_c| readiness
            for c, gate in ((1, (asem, A_ABS1)), (0, (vsem, V_AB0)),
                            (2, (asem, A_ABS2))):
                last = c == 2
                T.emit(lambda c=c: te.matmul(
                    p_y2[:, :], ab[:, c, :],
                    s_wb[:, B_W5AB + c * 128:B_W5AB + (c + 1) * 128],
                    start=False, stop=last, skip_group_check=True),
                    waits=[gate])
                T.emit(lambda c=c: te.matmul(
                    p_rm[:, 0:2], ab[:, c, :],
                    s_wb[:, B_RM_AB + 2 * c:B_RM_AB + 2 * c + 2],
                    start=False, stop=last, skip_group_check=True))
            assert T.n == P_RMAB2

        @block.vector
        def _(ve):
            V = _Seq(ve, vsem, validation)
            # elu front: r2 = relu(Wh) on DVE while ACT computes exp(Wh)
            V.emit(lambda: ve.tensor_scalar_max(out=r_[:, :],
                                                in0=p_y1[0][:, :],
                                                scalar1=0.0),
                   waits=[(psem, P_WHB)])
            assert V.n == V_R2
            # t1 = min(exp(Wh),1) + relu(Wh); s1 = sum(t1)
            V.emit(lambda: ve.scalar_tensor_tensor(out=t1[:, :], in0=ex[:, :],
                                                   scalar=1.0, in1=r_[:, :],
                                                   op0=Alu.min, op1=Alu.add,
                                                   accum_out=s1[:, :]),
                   waits=[(asem, A_EX)])
            assert V.n == V_T1
            # t2 = t1 - s1/128  (imm+AP tensor_scalar is broken on HW, so
            # scale the sum in a tiny op first)
            V.emit(lambda: ve.tensor_scalar_mul(out=s1m[:, :], in0=s1[:, :],
                                                scalar1=1.0 / 128.0),
                   self_wait=True)
            assert V.n == V_S1M
            V.emit(lambda: ve.tensor_scalar_sub(out=t2[:, :], in0=t1[:, :],
                                                scalar1=s1m[:, 0:1]),
                   self_wait=True)
            assert V.n == V_T2
            V.emit(lambda: ve.tensor_copy(out=t2T[:, :], in_=p_tT[:, :]),
                   waits=[(psem, P_T2T)])
            assert V.n == V_T2T
            # var1 path (only feeds the eps corrections; off critical path)
            V.emit(lambda: ve.bn_stats(out=st[:, :], in_=t1[:, :]))
            V.emit(lambda: ve.bn_aggr(out=mv[:, :], in_=st[:, :]),
                   self_wait=True)
            assert V.n == V_MV1
            V.emit(lambda: ve.tensor_scalar_add(out=v1[:, :], in0=mv[:, 1:2],
                                                scalar1=EPS))
            assert V.n == V_V1
            # leaky(ff): t3 = ff - 0.8*min(ff,0); s2 = sum(t3)
            # (one PSUM operand per instruction -- hardware restriction)
            V.emit(lambda: ve.tensor_scalar(out=lka[:, :], in0=p_q2[:, :],
                                            scalar1=0.0, scalar2=-0.8,
                                            op0=Alu.min, op1=Alu.mult),
                   waits=[(psem, P_FF)])
            V.emit(lambda: ve.scalar_tensor_tensor(
                out=t3[:, :], in0=lka[:, :], scalar=1.0, in1=p_q2[:, :],
                op0=Alu.mult, op1=Alu.add, accum_out=s2[:, :]))
            assert V.n == V_T3
            # u = t3 - s2/128
            V.emit(lambda: ve.tensor_scalar_mul(out=s2m[:, :], in0=s2[:, :],
                                                scalar1=1.0 / 128.0),
                   self_wait=True)
            assert V.n == V_S2M
            V.emit(lambda: ve.tensor_scalar_sub(out=u[:, :], in0=t3[:, :],
                                                scalar1=s2m[:, 0:1]),
                   self_wait=True)
            assert V.n == V_U
            V.emit(lambda: ve.tensor_copy(out=uT[:, :], in_=p_tT[:, :]),
                   waits=[(psem, P_UT)])
            assert V.n == V_UT
            # var2 path (off critical path, during PE wl)
            V.emit(lambda: ve.bn_stats(out=st[:, :], in_=t3[:, :]))
            V.emit(lambda: ve.bn_aggr(out=mv[:, :], in_=st[:, :]),
                   self_wait=True)
            assert V.n == V_MV2
            V.emit(lambda: ve.scalar_tensor_tensor(out=o_sb[:, 3:4],
                                                   in0=v1[:, :], scalar=EPS_K,
                                                   in1=mv[:, 1:2],
                                                   op0=Alu.mult, op1=Alu.add))
            assert V.n == V_V2
            # |M_0| on DVE (chunks 1,2 on ACT)
            V.emit(lambda: ve.tensor_scalar(out=y1a[:, :], in0=p_y1[0][:, :],
                                            scalar1=0.0, scalar2=-0.8,
                                            op0=Alu.min, op1=Alu.mult),
                   waits=[(psem, P_WL[0])])
            assert V.n == V_AB0A
            V.emit(lambda: ve.scalar_tensor_tensor(out=ab[:, 0, :],
                                                   in0=y1a[:, :], scalar=1.0,
                                                   in1=p_y1[0][:, :],
                                                   op0=Alu.mult, op1=Alu.add))
            assert V.n == V_AB0
            V.emit(lambda: ve.tensor_copy(out=o_sb[:, 0:2], in_=p_rm[:, 0:2]),
                   waits=[(psem, P_RMAB2)])
            assert V.n == V_REDC

    return nc, ctx


def _get_nc(validation=False):
    key = "ncv" if validation else "nc"
    if key not in _CACHE:
        _CACHE[key] = _build_nc(validation)
    return _CACHE[key][0]


_POST = {}


def _prep_in_maps(inputs):
    """Host-side sharding + exact algebraic weight folding + packing."""
    g = lambda k: np.asarray(inputs[k], dtype=np.float64)
    x = g("x")
    ei = np.asarray(inputs["edge_index"]).astype(np.int64)
    W = g("W")
    ff_w, ff_b = g("ff_w"), g("ff_b")
    na_g, na_b = g("na_g"), g("na_b")
    nf_g, nf_b = g("nf_g"), g("nf_b")
    wl_w, wl_b = g("wl_w"), g("wl_b")
    w5_w, w5_b = g("w5_w"), g("w5_b")
    fn_g, fn_b = g("fn_g"), g("fn_b")
    wv_w, wv_b = g("wv_w"), g("wv_b")

    xj = x[ei[1]]                           # [E, D] gather on host
    ffw_eff = ff_w * na_g[None, :]          # fold LN(na) gain into ff
    ffb_eff = ff_b + ff_w @ na_b
    wv_eff = wv_w[0] * fn_g                 # fold LN(fn) gain into wv
    wvb_eff = wv_b[0] + wv_w[0] @ fn_b
    wl_eff = wl_w * nf_g[None, :]           # fold LN(nf) gain into wl

    # the kernel structure assumes these vanish (true for the given inputs)
    assert np.all(ffb_eff == 0), "ffb_eff != 0 unsupported"
    assert np.all(wl_b == 0) and np.all(w5_b == 0), "wl/w5 bias unsupported"
    assert np.all(nf_b == 0), "nf_b != 0 unsupported"
    assert abs(wvb_eff) < 1e-12, "wvb != 0 unsupported"

    _POST["swv"] = float(wv_eff.sum())

    f16 = lambda a: np.ascontiguousarray(a, dtype=np.float16)

    wa = np.zeros((128, 256), np.float64)
    wa[:, A_ID:A_ID + 128] = np.eye(128)
    wa[:, A_FFWT:A_FFWT + 128] = ffw_eff.T

    wb = np.zeros((128, B_COLS), np.float64)
    wb[:, B_WLT:B_WLT + 384] = wl_eff.T
    # chunk 0 feeds exact leaky(M_0) into w5_0; chunks 1,2 are split as
    # 0.6*(w5_c@wl_c)@u + 0.4*w5_c@|M_c| (the |.| runs on the ACT engine)
    f16r = lambda a: a.astype(np.float16).astype(np.float64)
    for c, sc in ((0, 1.0), (1, 0.4), (2, 0.4)):
        wb[:, B_W5AB + c * 128:B_W5AB + (c + 1) * 128] = \
            sc * w5_w.T[c * 128:(c + 1) * 128, :]
    wb[:, B_Y2LIN:B_Y2LIN + 128] = \
        0.6 * (w5_w[:, 128:384] @ wl_eff[128:384, :]).T
    # red0/m3 columns: wv (resp. ones) pushed through the same matrices the
    # kernel actually uses (fp16-rounded), so red0 = sum(y3*wv), m3s = sum(y3)
    wv16 = f16r(wv_eff)
    ones = np.ones(128)
    rhs_lin = f16r(wb[:, B_Y2LIN:B_Y2LIN + 128])
    wb[:, B_RM_U] = wv16 + rhs_lin @ wv16
    wb[:, B_RM_U + 1] = ones + rhs_lin @ ones
    for c in range(3):
        rhs_ab = f16r(wb[:, B_W5AB + c * 128:B_W5AB + (c + 1) * 128])
        wb[:, B_RM_AB + 2 * c] = rhs_ab @ wv16
        wb[:, B_RM_AB + 2 * c + 1] = rhs_ab @ ones

    shared = {"wpacka": f16(wa), "wpackb": f16(wb)}
    in_maps = []
    for c in range(NCORES):
        xw = np.empty((128, 256), np.float64)
        xw[:, XW_XJT:XW_XJT + 128] = xj[c * PER:(c + 1) * PER].T
        xw[:, XW_W:XW_W + 128] = W
        m = dict(shared)
        m["xw"] = f16(xw)
        in_maps.append(m)
    return in_maps


def _postprocess_core(out_img):
    """[PER,4] (red0|mean3|var3|v2) -> [PER*D] final output."""
    o = np.asarray(out_img, dtype=np.float64).reshape(PER, 4)
    red0, m3s, sq3, v2 = o[:, 0], o[:, 1], o[:, 2], o[:, 3]
    m3 = m3s / 128.0
    var3 = sq3 / 128.0 - m3 * m3
    v3 = var3 + EPS_K * v2
    oe = (red0 - m3 * _POST["swv"]) / np.sqrt(v3)
    return np.repeat(oe.astype(np.float32), D)


def kernel(**inputs) -> np.ndarray:
    from concourse.bass_utils import run_bass_kernel_spmd

    nc = _get_nc()
    in_maps = _prep_in_maps(inputs)
    res = run_bass_kernel_spmd(nc, in_maps, core_ids=list(range(NCORES)))
    return np.concatenate(
        [_postprocess_core(res.results[c]["out"]) for c in range(NCORES)])


# revision 24
# speedup vs baseline: 1.0177x; 1.0146x over previous
"""Trainium2 Bass kernel for nn_AdjacencyGenerator (gnn_message_passing).

Math note (see kernel_baseline.py for the original derivation): softmax over
dim 1 of the [E,E,D] attention tensor sums to 1, so the attention cancels and
the output is a per-edge scalar o[i] = f(Wh[i,:]) repeated D times, where
  f: elu -> LN(na) -> ff -> leaky -> LN(nf) -> wl -> leaky -> w5 -> +res
     -> LN(fn) -> wv.

Beyond the baseline, this version exploits:
  * scale invariance: LN_core(a*x) = LN_core(x) for per-row a>0, and all the
    layers between LNs are positively homogeneous.  No rstd is ever applied
    on-chip; the three factors collapse into one final rsqrt via
        v1 = var1 + eps,  v2 = var2 + eps*128^2*v1,  v3 = var3 + eps*128^2*v2
        out[e] = (red0[e] - mean3[e]*sum(wv_eff)) * rsqrt(v3[e])
    computed on the HOST from 4 shipped scalars per edge (exact algebra; the
    128^2 factors come from the mean-sub trick below).
  * mean subtraction via the accumulator: the op producing each LN input also
    emits its row-sum s, and the centering is one op: x' = 128*x - s
    (the extra 128 scale is absorbed by scale invariance).
  * elu(x)+1 = min(exp(x),1) + relu(x): exp runs on ACT straight from PSUM
    while DVE computes the relu part in parallel.
  * leaky_0.2(x) = 0.6*x + 0.4*|x|: wl chunks 1,2 use one ACT Abs + one DVE
    op (0.6 folded into w5); chunk 0 stays DVE-only for pipeline balance.
  * fp16 everywhere on the PE path, including fp16 PSUM banks for the
    single-shot matmuls (halves the DVE PSUM-read cost).
  * the final wv dot product is 4 tiny PE matmuls (wv folded through w5)
    accumulating into a PSUM column, not a DVE reduction.

Distribution: 1024 edges, 128 per core across 8 cores, weights replicated.
"""

import numpy as np

D = 128
E = 1024
NCORES = 8
PER = E // NCORES
EPS = 1e-5
EPS_K = EPS                   # mean-subs are exact, no extra scale

# packed image column offsets (fp16)
XW_XJT, XW_W = 0, 128                       # d_xw [128, 256]
A_ID, A_FFWT = 0, 128                       # d_wa [128, 256]
B_WLT, B_W5AB, B_Y2LIN = 0, 384, 768    # d_wb [128, 904]
B_RM_U, B_RM_AB = 896, 898              # [wv|ones] column pairs
B_COLS = 904

_CACHE = {}


class _Seq:
    """Sequential instruction emitter for one engine with semaphore tags."""

    def __init__(self, eng, sem, all_self_waits, attach=False):
        self.eng, self.sem, self.n = eng, sem, 0
        self.all_self_waits = all_self_waits
        self.attach = attach

    def emit(self, make, waits=(), self_wait=False):
        allw = list(waits)
        if (self_wait or self.all_self_waits) and self.n:
            allw.append((self.sem, self.n))
        if self.attach and allw:
            for s, v in allw[:-1]:
                self.eng.wait_ge(s, v)
            inst = make()
            inst._wait_ge(*allw[-1])
        else:
            for s, v in allw:
                self.eng.wait_ge(s, v)
            inst = make()
        inst.then_inc(self.sem, 1)
        self.n += 1
        return self.n


def _build_nc(validation=False):
    import concourse.bass as bass
    from concourse import mybir

    f32 = mybir.dt.float32
    f16 = mybir.dt.float16
    Alu = mybir.AluOpType
    Act = mybir.ActivationFunctionType

    nc = bass.Bass(detect_race_conditions=validation)

    d_xw = nc.dram_tensor("xw", [128, 256], f16, kind="ExternalInput")
    d_wa = nc.dram_tensor("wpacka", [128, 256], f16, kind="ExternalInput")
    d_wb = nc.dram_tensor("wpackb", [128, B_COLS], f16, kind="ExternalInput")
    d_out = nc.dram_tensor("out", [PER, 4], f32, kind="ExternalOutput")

    from contextlib import ExitStack

    ctx = ExitStack()
    sb = lambda name, shape, dt=f32: ctx.enter_context(
        nc.sbuf_tensor(name, shape, dt))
    ps = lambda name, shape, dt=f32: ctx.enter_context(
        nc.psum_tensor(name, shape, dt))

    s_xw = sb("s_xw", [128, 256], f16)
    s_wa = sb("s_wa", [128, 256], f16)
    s_wb = sb("s_wb", [128, B_COLS], f16)

    r_ = sb("r", [PER, D], f16)        # relu(Wh)
    ex = sb("ex", [PER, D], f16)       # exp(Wh)
    t1 = sb("t1", [PER, D], f16)       # elu(Wh)+1
    s1 = sb("s1", [PER, 1])            # sum(t1)
    t2 = sb("t2", [PER, D], f16)       # 128*t1 - s1
    t2T = sb("t2t", [D, PER], f16)
    lka = sb("lka", [PER, D], f16)     # ff out staging
    t3 = sb("t3", [PER, D], f16)       # leaky(ff)
    s2 = sb("s2", [PER, 1])            # sum(t3)
    u = sb("u", [PER, D], f16)         # 128*t3 - s2
    uT = sb("ut", [D, PER], f16)
    ab = sb("ab", [128, 3, PER], f16)  # leaky(wl_0) / |wl_1| / |wl_2|
    y1a = sb("y1a", [128, PER], f16)   # -0.8*min(wl_0, 0) scratch
    sq = sb("sq", [PER, D], f16)       # y3^2 scratch
    s1m = sb("s1m", [PER, 1])          # mean1
    s2m = sb("s2m", [PER, 1])          # mean2
    st = sb("st", [PER, 6])
    mv = sb("mv", [PER, 2])
    v1 = sb("v1", [PER, 1])
    o_sb = sb("o_sb", [PER, 4])        # red0 | mean3 | var3 | v2
    scr = sb("scr", [1, 1])            # ACT warmup scratch

    p_wh = ps("p_wh", [PER, D])
    p_tT = ps("p_tt", [D, PER], f16)   # reused for t2T and uT
    p_q2 = ps("p_q2", [PER, D])
    p_y1 = [ps(f"p_y1{c}", [128, PER]) for c in range(3)]
    p_y2 = ps("p_y2", [PER, D])
    p_rm = ps("p_rm", [PER, 2])       # col0: sum(y3*wv), col1: sum(y3)

    dsem_x = ctx.enter_context(nc.semaphore("dsem_x"))
    dsem_a = ctx.enter_context(nc.semaphore("dsem_a"))
    dsem_b = ctx.enter_context(nc.semaphore("dsem_b"))
    dsem_o = ctx.enter_context(nc.semaphore("dsem_o"))
    psem = ctx.enter_context(nc.semaphore("psem"))
    vsem = ctx.enter_context(nc.semaphore("vsem"))
    asem = ctx.enter_context(nc.semaphore("asem"))
    gsem = ctx.enter_context(nc.semaphore("gsem"))

    # ---- vector op indices ----------------------------------------------
    V_R2, V_T1, V_S1M, V_T2, V_T2T = 1, 2, 3, 4, 5
    V_ST1, V_MV1, V_V1 = 6, 7, 8
    V_LKA, V_T3, V_S2M, V_U, V_UT = 9, 10, 11, 12, 13
    V_ST2, V_MV2, V_V2 = 14, 15, 16
    V_AB0A, V_AB0, V_REDC = 17, 18, 19
    # ---- PE op indices ---------------------------------------------------
    P_WH, P_WHB, P_T2T, P_FF, P_UT = 1, 2, 3, 4, 5
    P_WL = [6, 7, 8]
    P_RES, P_Y2LIN, P_RMU = 9, 10, 11
    P_AB0, P_RMAB0 = 12, 13
    P_AB1, P_RMAB1 = 14, 15
    P_AB2, P_RMAB2 = 16, 17
    # ---- ACT op indices --------------------------------------------------
    A_WARM, A_EX, A_ABS1, A_ABS2, A_SQ3 = 1, 2, 3, 4, 5
    # ---- gpsimd ----------------------------------------------------------
    G_SCR = 1

    with nc.Block() as block:

        @block.sync
        def _(sync):
            sync.dma_start(out=s_xw[:, :], in_=d_xw[:, :]).then_inc(dsem_x, 16)
            sync.dma_start(out=s_wa[:, :], in_=d_wa[:, :]).then_inc(dsem_a, 16)
            sync.dma_start(out=s_wb[:, :], in_=d_wb[:, :]).then_inc(dsem_b, 16)
            sync.wait_ge(vsem, V_REDC)
            sync.wait_ge(asem, A_SQ3)
            sync.dma_start(out=d_out[:, :], in_=o_sb[:, :]).then_inc(dsem_o, 16)

        @block.gpsimd
        def _(ge):
            ge.memset(scr[:, :], 1.0).then_inc(gsem, 1)

        @block.scalar
        def _(se):
            A = _Seq(se, asem, validation)
            # warm the ln/exp table set (Exp/Abs share it)
            A.emit(lambda: se.activation(out=scr[:, :], in_=scr[:, :],
                                         func=Act.Ln),
                   waits=[(gsem, G_SCR)])
            A.emit(lambda: se.activation(out=ex[:, :], in_=p_wh[:, :],
                                         func=Act.Exp),
                   waits=[(psem, P_WH)])
            assert A.n == A_EX
            # |wl_1|, |wl_2| on ACT; chunk 0 is exact leaky on DVE
            A.emit(lambda: se.activation(out=ab[:, 1, :], in_=p_y1[1][:, :],
                                         func=Act.Abs),
                   waits=[(psem, P_WL[1])])
            assert A.n == A_ABS1
            A.emit(lambda: se.activation(out=ab[:, 2, :], in_=p_y1[2][:, :],
                                         func=Act.Abs),
                   waits=[(psem, P_WL[2])])
            assert A.n == A_ABS2
            # sum(y3^2) via the ACT accumulator, straight off the closed PSUM
            A.emit(lambda: se.activation(out=sq[:, :], in_=p_y2[:, :],
                                         func=Act.Square,
                                         accum_out=o_sb[:, 2:3]),
                   waits=[(psem, P_AB2)])
            assert A.n == A_SQ3

        @block.tensor
        def _(te):
            T = _Seq(te, psem, validation)
            # Wh = xj @ W
            # Wh twice, into two banks: ACT reads p_wh while DVE reads the
            # copy in p_y1[0]'s bank (same-bank dual-engine PSUM reads hang
            # the device; p_y1[0] is dead until the wl matmuls)
            T.emit(lambda: te.matmul(p_wh[:, :], s_xw[:, XW_XJT:XW_XJT + 128],
                                     s_xw[:, XW_W:XW_W + 128],
                                     start=True, stop=True),
                   waits=[(dsem_x, 16)])
            T.emit(lambda: te.matmul(p_y1[0][:, :],
                                     s_xw[:, XW_XJT:XW_XJT + 128],
                                     s_xw[:, XW_W:XW_W + 128],
                                     start=True, stop=True))
            assert T.n == P_WHB
            T.emit(lambda: te.transpose(p_tT[:, :], t2[:, :],
                                        s_wa[:, A_ID:A_ID + 128]),
                   waits=[(vsem, V_T2), (dsem_a, 16)])
            assert T.n == P_T2T
            T.emit(lambda: te.matmul(p_q2[:, :], t2T[:, :],
                                     s_wa[:, A_FFWT:A_FFWT + 128],
                                     start=True, stop=True),
                   waits=[(vsem, V_T2T)])
            T.emit(lambda: te.transpose(p_tT[:, :], u[:, :],
                                        s_wa[:, A_ID:A_ID + 128]),
                   waits=[(vsem, V_U)])
            assert T.n == P_UT
            # wl chunks: M_c = wl_c @ u^T
            for c in range(3):
                T.emit(lambda c=c: te.matmul(
                    p_y1[c][:, :],
                    s_wb[:, B_WLT + c * 128:B_WLT + (c + 1) * 128],
                    uT[:, :], start=True, stop=True),
                    waits=[(vsem, V_UT), (dsem_b, 16)] if c == 0 else ())
                assert T.n == P_WL[c]
            # y3 = u + 0.6*(w5@wl)@u + 0.4*sum_c w5_c@|M_c|  (leaky split);
            # p_red/p_m3 accumulate sum(y3*wv) and sum(y3) the same way
            T.emit(lambda: te.matmul(p_y2[:, :], uT[:, :],
                                     s_wa[:, A_ID:A_ID + 128],
                                     start=True, stop=False,
                                     skip_group_check=True))
            assert T.n == P_RES
            T.emit(lambda: te.matmul(p_y2[:, :], uT[:, :],
                                     s_wb[:, B_Y2LIN:B_Y2LIN + 128],
                                     start=False, stop=False,
                                     skip_group_check=True))
            assert T.n == P_Y2LIN
            T.emit(lambda: te.matmul(p_rm[:, 0:2], uT[:, :],
                                     s_wb[:, B_RM_U:B_RM_U + 2],
                                     start=True, stop=False,
                                     skip_group_check=True))
            assert T.n == P_RMU
            # abs-consuming matmuls, in expected order of |M_c| readiness
            for c, gate in ((1, (asem, A_ABS1)), (0, (vsem, V_AB0)),
                            (2, (asem, A_ABS2))):
                last = c == 2
                T.emit(lambda c=c: te.matmul(
                    p_y2[:, :], ab[:, c, :],
                    s_wb[:, B_W5AB + c * 128:B_W5AB + (c + 1) * 128],
                    start=False, stop=last, skip_group_check=True),
                    waits=[gate])
                T.emit(lambda c=c: te.matmul(
                    p_rm[:, 0:2], ab[:, c, :],
                    s_wb[:, B_RM_AB + 2 * c:B_RM_AB + 2 * c + 2],
                    start=False, stop=last, skip_group_check=True))
            assert T.n == P_RMAB2

        @block.vector
        def _(ve):
            V = _Seq(ve, vsem, validation)
            # elu front: r2 = relu(Wh) on DVE while ACT computes exp(Wh)
            V.emit(lambda: ve.tensor_scalar_max(out=r_[:, :],
                                                in0=p_y1[0][:, :],
                                                scalar1=0.0),
                   waits=[(psem, P_WHB)])
            assert V.n == V_R2
            # t1 = min(exp(Wh),1) + relu(Wh); s1 = sum(t1)
            V.emit(lambda: ve.scalar_tensor_tensor(out=t1[:, :], in0=ex[:, :],
                                                   scalar=1.0, in1=r_[:, :],
                                                   op0=Alu.min, op1=Alu.add,
                                                   accum_out=s1[:, :]),
                   waits=[(asem, A_EX)])
            assert V.n == V_T1
            # t2 = t1 - s1/128  (imm+AP tensor_scalar is broken on HW, so
            # scale the sum in a tiny op first)
            V.emit(lambda: ve.tensor_scalar_mul(out=s1m[:, :], in0=s1[:, :],
                                                scalar1=1.0 / 128.0),
                   self_wait=True)
            assert V.n == V_S1M
            V.emit(lambda: ve.tensor_scalar_sub(out=t2[:, :], in0=t1[:, :],
                                                scalar1=s1m[:, 0:1]),
                   self_wait=True)
            assert V.n == V_T2
            V.emit(lambda: ve.tensor_copy(out=t2T[:, :], in_=p_tT[:, :]),
                   waits=[(psem, P_T2T)])
            assert V.n == V_T2T
            # var1 path (only feeds the eps corrections; off critical path)
            V.emit(lambda: ve.bn_stats(out=st[:, :], in_=t1[:, :]))
            V.emit(lambda: ve.bn_aggr(out=mv[:, :], in_=st[:, :]),
                   self_wait=True)
            assert V.n == V_MV1
            V.emit(lambda: ve.tensor_scalar_add(out=v1[:, :], in0=mv[:, 1:2],
                                                scalar1=EPS))
            assert V.n == V_V1
            # leaky(ff) = max(q2, 0.2*q2); copy PSUM->SBUF first (only one
            # PSUM operand per instruction is allowed in hardware)
            V.emit(lambda: ve.tensor_copy(out=lka[:, :], in_=p_q2[:, :]),
                   waits=[(psem, P_FF)])
            V.emit(lambda: ve.scalar_tensor_tensor(
                out=t3[:, :], in0=lka[:, :], scalar=0.2, in1=lka[:, :],
                op0=Alu.mult, op1=Alu.max, accum_out=s2[:, :]))
            assert V.n == V_T3
            # u = t3 - s2/128
            V.emit(lambda: ve.tensor_scalar_mul(out=s2m[:, :], in0=s2[:, :],
                                                scalar1=1.0 / 128.0),
                   self_wait=True)
            assert V.n == V_S2M
            V.emit(lambda: ve.tensor_scalar_sub(out=u[:, :], in0=t3[:, :],
                                                scalar1=s2m[:, 0:1]),
                   self_wait=True)
            assert V.n == V_U
            V.emit(lambda: ve.tensor_copy(out=uT[:, :], in_=p_tT[:, :]),
                   waits=[(psem, P_UT)])
            assert V.n == V_UT
            # var2 path (off critical path, during PE wl)
            V.emit(lambda: ve.bn_stats(out=st[:, :], in_=t3[:, :]))
            V.emit(lambda: ve.bn_aggr(out=mv[:, :], in_=st[:, :]),
                   self_wait=True)
            assert V.n == V_MV2
            V.emit(lambda: ve.scalar_tensor_tensor(out=o_sb[:, 3:4],
                                                   in0=v1[:, :], scalar=EPS_K,
                                                   in1=mv[:, 1:2],
                                                   op0=Alu.mult, op1=Alu.add))
            assert V.n == V_V2
            # |M_0| on DVE (chunks 1,2 on ACT)
            V.emit(lambda: ve.tensor_copy(out=y1a[:, :], in_=p_y1[0][:, :]),
                   waits=[(psem, P_WL[0])])
            assert V.n == V_AB0A
            V.emit(lambda: ve.scalar_tensor_tensor(out=ab[:, 0, :],
                                                   in0=y1a[:, :], scalar=0.2,
                                                   in1=y1a[:, :],
                                                   op0=Alu.mult, op1=Alu.max))
            assert V.n == V_AB0
            V.emit(lambda: ve.tensor_copy(out=o_sb[:, 0:2], in_=p_rm[:, 0:2]),
                   waits=[(psem, P_RMAB2)])
            assert V.n == V_REDC

    return nc, ctx


def _get_nc(validation=False):
    key = "ncv" if validation else "nc"
    if key not in _CACHE:
        _CACHE[key] = _build_nc(validation)
    return _CACHE[key][0]


_POST = {}


def _prep_in_maps(inputs):
    """Host-side sharding + exact algebraic weight folding + packing."""
    g = lambda k: np.asarray(inputs[k], dtype=np.float64)
    x = g("x")
    ei = np.asarray(inputs["edge_index"]).astype(np.int64)
    W = g("W")
    ff_w, ff_b = g("ff_w"), g("ff_b")
    na_g, na_b = g("na_g"), g("na_b")
    nf_g, nf_b = g("nf_g"), g("nf_b")
    wl_w, wl_b = g("wl_w"), g("wl_b")
    w5_w, w5_b = g("w5_w"), g("w5_b")
    fn_g, fn_b = g("fn_g"), g("fn_b")
    wv_w, wv_b = g("wv_w"), g("wv_b")

    xj = x[ei[1]]                           # [E, D] gather on host
    ffw_eff = ff_w * na_g[None, :]          # fold LN(na) gain into ff
    ffb_eff = ff_b + ff_w @ na_b
    wv_eff = wv_w[0] * fn_g                 # fold LN(fn) gain into wv
    wvb_eff = wv_b[0] + wv_w[0] @ fn_b
    wl_eff = wl_w * nf_g[None, :]           # fold LN(nf) gain into wl

    # the kernel structure assumes these vanish (true for the given inputs)
    assert np.all(ffb_eff == 0), "ffb_eff != 0 unsupported"
    assert np.all(wl_b == 0) and np.all(w5_b == 0), "wl/w5 bias unsupported"
    assert np.all(nf_b == 0), "nf_b != 0 unsupported"
    assert abs(wvb_eff) < 1e-12, "wvb != 0 unsupported"

    _POST["swv"] = float(wv_eff.sum())

    f16 = lambda a: np.ascontiguousarray(a, dtype=np.float16)

    wa = np.zeros((128, 256), np.float64)
    wa[:, A_ID:A_ID + 128] = np.eye(128)
    wa[:, A_FFWT:A_FFWT + 128] = ffw_eff.T

    wb = np.zeros((128, B_COLS), np.float64)
    wb[:, B_WLT:B_WLT + 384] = wl_eff.T
    # chunk 0 feeds exact leaky(M_0) into w5_0; chunks 1,2 are split as
    # 0.6*(w5_c@wl_c)@u + 0.4*w5_c@|M_c| (the |.| runs on the ACT engine)
    f16r = lambda a: a.astype(np.float16).astype(np.float64)
    for c, sc in ((0, 1.0), (1, 0.4), (2, 0.4)):
        wb[:, B_W5AB + c * 128:B_W5AB + (c + 1) * 128] = \
            sc * w5_w.T[c * 128:(c + 1) * 128, :]
    wb[:, B_Y2LIN:B_Y2LIN + 128] = \
        0.6 * (w5_w[:, 128:384] @ wl_eff[128:384, :]).T
    # red0/m3 columns: wv (resp. ones) pushed through the same matrices the
    # kernel actually uses (fp16-rounded), so red0 = sum(y3*wv), m3s = sum(y3)
    wv16 = f16r(wv_eff)
    ones = np.ones(128)
    rhs_lin = f16r(wb[:, B_Y2LIN:B_Y2LIN + 128])
    wb[:, B_RM_U] = wv16 + rhs_lin @ wv16
    wb[:, B_RM_U + 1] = ones + rhs_lin @ ones
    for c in range(3):
        rhs_ab = f16r(wb[:, B_W5AB + c * 128:B_W5AB + (c + 1) * 128])
        wb[:, B_RM_AB + 2 * c] = rhs_ab @ wv16
        wb[:, B_RM_AB + 2 * c + 1] = rhs_ab @ ones

    shared = {"wpacka": f16(wa), "wpackb": f16(wb)}
    in_maps = []
    for c in range(NCORES):
        xw = np.empty((128, 256), np.float64)
        xw[:, XW_XJT:XW_XJT + 128] = xj[c * PER:(c + 1) * PER].T
        xw[:, XW_W:XW_W + 128] = W
        m = dict(shared)
        m["xw"] = f16(xw)
        in_maps.append(m)
    return in_maps


def _postprocess_core(out_img):
    """[PER,4] (red0|mean3|var3|v2) -> [PER*D] final output."""
    o = np.asarray(out_img, dtype=np.float64).reshape(PER, 4)
    red0, m3s, sq3, v2 = o[:, 0], o[:, 1], o[:, 2], o[:, 3]
    m3 = m3s / 128.0
    var3 = sq3 / 128.0 - m3 * m3
    v3 = var3 + EPS_K * v2
    oe = (red0 - m3 * _POST["swv"]) / np.sqrt(v3)
    return np.repeat(oe.astype(np.float32), D)


def kernel(**inputs) -> np.ndarray:
    from concourse.bass_utils import run_bass_kernel_spmd

    nc = _get_nc()
    in_maps = _prep_in_maps(inputs)
    res = run_bass_kernel_spmd(nc, in_maps, core_ids=list(range(NCORES)))
    return np.concatenate(
        [_postprocess_core(res.results[c]["out"]) for c in range(NCORES)])
